# revision 3
# baseline (speedup 1.0000x reference)
"""Trainium2 Bass kernel for nn_Decoder_34325378630277 (FNO-UNet decoder).

Sharding: 8 cores = 2 batches x 4 row-quarters (64 owned rows each).
Conv halos handled by extended recompute (host supplies zero-padded row
slices).

v2 design:
- FNO forward spectral projection is fused into the conv stage that
  produces the FNO source: the source is stored DENSE (c, r*256), and
  per 8-row group an SBUF->SBUF XBAR dma-transpose produces (w, blk, c)
  slabs that feed accumulating matmuls into a per-channel spectrum
  pxfT (Ci, 32) held in PSUM across the stage.  This removes the
  DRAM-bounce transposes (which were both slow and racy).
- The spectrum AllReduce fires right at conv end, hidden behind the
  1x1-conv pass A; the channel mix happens in (Ci, m) orientation so no
  spectrum transposes are needed.
- Pass A/B use 4-row chunks (2-bank PSUM tiles), with the gelu on the
  scalar engine, the spectral add on vector, and the skip add on the
  otherwise-idle Pool (gpsimd) engine.
- Everything bf16 except PSUM accumulation and the final output.
"""
import sys, types

sys.path.insert(0, "/opt/trn_rl_repo")
import numpy as np
import ml_dtypes

# NTFF profile hook shim (lets trace=True work under axon; harmless otherwise)
try:
    import antenv  # noqa: F401
    if "antenv.axon_hooks" not in sys.modules:
        _h = {"hook": None}
        _m = types.ModuleType("antenv.axon_hooks")
        _m.set_axon_ntff_profile_hook = lambda h: _h.__setitem__("hook", h)
        _m.get_axon_ntff_profile_hook = lambda: _h["hook"]
        sys.modules["antenv.axon_hooks"] = _m
        from trn_agent_boot.trn_boot import _ntff_profile_via_ctypes
        _m.set_axon_ntff_profile_hook(
            _ntff_profile_via_ctypes("/opt/axon/libaxon_pjrt.so"))
except Exception:
    pass

import concourse.bass as bass
import concourse.bacc as bacc
import concourse.tile as tile
from concourse import mybir, masks
from concourse.bass_utils import run_bass_kernel_spmd

F32 = mybir.dt.float32
F16 = mybir.dt.float16
AF = mybir.ActivationFunctionType

B, HH, WW, NF = 2, 256, 256, 16
OWN = 64
NCORES = 8
WP = WW + 2          # padded width (zero cols at 0 and WP-1)
EMAX = 9             # x5u halo
M = 4                # modes kept per axis
GR = 8               # row-group granularity for streaming loads
TGR = 8              # transpose group rows (fused fwd projection)

# fno stages: (tag, Ci, Co, e)
FNOS = [("f5", 256, 128, 9), ("f6", 128, 64, 6), ("f7", 64, 32, 3), ("f8", 32, 16, 1)]
FNOD = {t: (ci, co, e) for t, ci, co, e in FNOS}
# conv blocks: (tag, C, n_stages, e_dst per stage)
CONVS = [("c6", 128, 3, [8, 7, 6]), ("c7", 64, 3, [5, 4, 3]),
         ("c8", 32, 2, [2, 1]), ("c9", 16, 1, [0])]
CWSHP = {"c6": (3, 9, 128, 128), "c7": (3, 6, 128, 64),
         "c8": (2, 3, 96, 32), "c9": (1, 3, 48, 16)}


def _rows(e):
    return OWN + 2 * e


# ---------------------------------------------------------------------------
# device program
# ---------------------------------------------------------------------------

def _build_nc():
    nc = bacc.Bacc("TRN2", target_bir_lowering=False, debug=False,
                   num_devices=NCORES)

    def din(name, shape, dt):
        return nc.dram_tensor(name, list(shape), dt, kind="ExternalInput").ap()

    # --- external inputs (per-core data) ---
    x5b_sl = din("x5b_sl", (256, _rows(9) * WW), F16)
    x4_sl = din("x4_sl", (128, _rows(9) * WW), F16)
    x3_sl = din("x3_sl", (64, _rows(6) * WW), F16)
    x2_sl = din("x2_sl", (32, _rows(3) * WW), F16)
    x1_sl = din("x1_sl", (16, _rows(1) * WW), F16)
    skips = {"f5": x4_sl, "f6": x3_sl, "f7": x2_sl, "f8": x1_sl}
    x5T = din("x5T", (2, OWN, 128, 256), F16)          # (wt, h, w, c)
    zpadb = din("zpadb", (128, 2 * _rows(9)), F16)     # zeros for pad cols
    fb_in = din("fb", (128, 2 * OWN * 32), F16)        # (w, (wt h m)) fwd basis
    gb = din("gb", (32, _rows(9) * WW), F16)           # inv basis rows r0-9..r0+73
    mask_pp = din("mask_pp", (128, _rows(9)), F16)     # in-image mask, replicated
    bsel = din("bsel", (128, 2), F32)                  # one-hot batch select
    cw = {}
    cb = {}
    for tag, C, nst, _ in CONVS:
        cw[tag] = din(tag + "w", CWSHP[tag], F16)
        cb[tag] = din(tag + "b", (128, nst), F32)
    bwT = {}
    bb = {}
    mixw = {}
    for tag, Ci, Co, _ in FNOS:
        kt = (Ci + 127) // 128
        bwT[tag] = din(tag + "_bwT", (128, kt * Co), F16)
        bb[tag] = din(tag + "_bb", (128, 1), F32)
        mixw[tag] = din(tag + "_mixw", (128, kt * 16 * 2 * Co), F16)
    owT = din("owT", (16, 2), F16)
    ob = din("ob", (2, 1), F32)

    out_sl = nc.dram_tensor("out_sl", [2, OWN * WW], F32,
                            kind="ExternalOutput").ap()

    # --- internal dram (collective buffers) ---
    cc_in = {}
    cc_out = {}
    for tag, Ci, Co, e in FNOS:
        kt = (Ci + 127) // 128
        cp = min(Ci, 128)
        cc_in[tag] = nc.dram_tensor("ccin_" + tag, [2, cp, kt * 32], F32).ap()
        cc_out[tag] = nc.dram_tensor("ccout_" + tag, [2, cp, kt * 32], F32,
                                     addr_space="Shared").ap()
    ccw_in = nc.dram_tensor("ccw_in", [1, 8], F32).ap()
    ccw_out = nc.dram_tensor("ccw_out", [1, 8], F32, addr_space="Shared").ap()

    with tile.TileContext(nc) as tc:
        import contextlib
        est = contextlib.ExitStack()
        with est:
            big = est.enter_context(tc.tile_pool(name="big", bufs=1))
            cst = est.enter_context(tc.tile_pool(name="cst", bufs=1))
            stm2 = est.enter_context(tc.tile_pool(name="stm2", bufs=2))
            stm3 = est.enter_context(tc.tile_pool(name="stm3", bufs=3))
            xtp = est.enter_context(tc.tile_pool(name="xtp", bufs=2))
            tmp1 = est.enter_context(tc.tile_pool(name="tmp1", bufs=1))
            ppool = est.enter_context(
                tc.tile_pool(name="ppool", bufs=3, space="PSUM"))
            fpsp = est.enter_context(
                tc.tile_pool(name="fpsp", bufs=2, space="PSUM"))
            pspec = est.enter_context(
                tc.tile_pool(name="pspec", bufs=1, space="PSUM"))

            ALL8 = [list(range(NCORES))]
            # warmup collective (absorbs communicator init early)
            nc.gpsimd.collective_compute(
                "AllReduce", mybir.AluOpType.add, replica_groups=ALL8,
                ins=[ccw_in[:]], outs=[ccw_out[:]])

            # --- constants resident in SBUF (split across the two HWDGE qs) ---
            ident = cst.tile([128, 128], F16, tag="ident")
            masks.make_identity(nc, ident[:])
            fb_t = cst.tile([128, 2 * OWN * 32], F16, tag="fb")
            nc.sync.dma_start(fb_t[:], fb_in[:])
            mask_t = cst.tile([128, _rows(9)], F16, tag="mask")
            nc.scalar.dma_start(mask_t[:], mask_pp[:])
            bsel_t = cst.tile([128, 2], F32, tag="bsel")
            nc.scalar.dma_start(bsel_t[:], bsel[:])

            def fb_sl(wt, h):  # (128, 32) lhsT slab for fwd basis
                o = (wt * OWN + h) * 32
                return fb_t[:, o:o + 32]

            bwT_t = {}
            bb_t = {}
            mixw_t = {}
            for i, (tag, Ci, Co, e) in enumerate(FNOS):
                kt = (Ci + 127) // 128
                q = nc.sync if tag == "f5" else nc.scalar
                bwT_t[tag] = cst.tile([128, kt * Co], F16, tag="bw" + tag,
                                      name="bw_" + tag)
                q.dma_start(bwT_t[tag][:], bwT[tag][:])
                bb_t[tag] = cst.tile([128, 1], F32, tag="bb" + tag,
                                     name="bbt_" + tag)
                q.dma_start(bb_t[tag][:], bb[tag][:])
                mixw_t[tag] = cst.tile([128, kt * 16 * 2 * Co], F16,
                                       tag="mw" + tag, name="mwt_" + tag)
                nc.scalar.dma_start(mixw_t[tag][:], mixw[tag][:])
            cb_t = {}
            cw_t = {}
            for i, (tag, C, nst, _) in enumerate(CONVS):
                q = nc.scalar
                cb_t[tag] = cst.tile([128, nst], F32, tag="cb" + tag,
                                     name="cbt_" + tag)
                q.dma_start(cb_t[tag][:], cb[tag][:])
                nt, kw = CWSHP[tag][1], CWSHP[tag][2]
                cw_t[tag] = []
                for st in range(nst):
                    w_t = cst.tile([128, nt * C], F16, tag=f"cw{tag}{st}",
                                   name=f"cwt_{tag}{st}")
                    q.dma_start(
                        w_t[:kw].rearrange("i (t o) -> i t o", t=nt),
                        cw[tag][st].rearrange("t i o -> i t o"))
                    cw_t[tag].append(w_t)
            ow_t = cst.tile([16, 2], F16, tag="ow")
            nc.scalar.dma_start(ow_t[:], owT[:])
            ob_t = cst.tile([2, 1], F32, tag="ob")
            nc.scalar.dma_start(ob_t[:], ob[:])

            # --- big activation slots (bf16) ---
            def new_act(slot, e):
                """Padded activation (128, R*WP) with zeroed pad columns."""
                t = big.tile([128, _rows(e) * WP], F16, tag=slot,
                             name="act_" + slot)
                R = _rows(e)
                z = t[:].rearrange("c (r w) -> c r w", w=WP)
                nc.scalar.dma_start(z[:, :, 0:1], zpadb[:, :R])
                nc.scalar.dma_start(z[:, :, WP - 1:WP], zpadb[:, R:2 * R])
                return t

            def new_dense(slot, C, R):
                """Dense activation (C, R*256), no pads (FNO sources, x9)."""
                return big.tile([128, R * 256], F16, tag=slot,
                                name="dact_" + slot)

            def act_view(t, C, e):
                return t[:C].rearrange("c (r w) -> c r w", w=WP)

            # per-chunk epilogue for padded dsts: boundary mask + stack copies
            qrot = [nc.sync, nc.scalar]

            def chunk_epilogue(dst_t, C, e, rd, cr, nstack, qi=0):
                R = _rows(e)
                moff = EMAX - e
                dv = act_view(dst_t, C, e)
                if rd < 9 or rd + cr > R - 9:
                    nc.vector.tensor_mul(
                        dv[:, rd:rd + cr, :], dv[:, rd:rd + cr, :],
                        mask_t[:C, moff + rd:moff + rd + cr]
                        .broadcast_to((C, cr, WP)))
                if nstack > 1:
                    v = dst_t[:].rearrange("c (r w) -> c r w", w=WP)
                    for k in range(1, nstack):
                        lo = max(0, rd - k)
                        hi = min(R - k, rd + cr - k)
                        if hi > lo:
                            qrot[(qi + k) % 2].dma_start(
                                v[k * C:(k + 1) * C, lo:hi, :],
                                v[0:C, lo + k:hi + k, :])

            # ---------- fused forward-projection state ----------
            class FwdState:
                def __init__(self, tag):
                    Ci, Co, e = FNOD[tag]
                    self.tag, self.Ci, self.e = tag, Ci, e
                    self.kt = (Ci + 127) // 128
                    cp = min(Ci, 128)
                    self.pxf = pspec.tile([cp, self.kt * 32], F32,
                                          tag="spec", name="pxf_" + tag)
                    self.i = [0] * self.kt      # per-chain matmul counter
                    self.n = 2 * OWN            # matmuls per chain
                    self.pending = None

                def emit_pending(self):
                    if self.pending is None:
                        return
                    xt3, olo, nrows = self.pending
                    self.pending = None
                    for j in range(nrows):
                        h = olo - self.e + j
                        for wt in range(2):
                            k = 0  # kt==1 for fused tags (f6/f7/f8)
                            nc.tensor.matmul(
                                self.pxf[:, 0:32],
                                xt3[:, 2 * j + wt, :],
                                fb_sl(wt, h),
                                start=(self.i[k] == 0),
                                stop=(self.i[k] == self.n - 1),
                                skip_group_check=True)
                            self.i[k] += 1

            # ---------------- conv stage ----------------
            def conv_stage(tag, C, st, e, src_t, src_e, dst_t,
                           dst_stack=1, dense=False, fwd=None):
                nt, kw = CWSHP[tag][1], CWSHP[tag][2]
                wsl = cw_t[tag][st]
                svf = src_t[:].rearrange("c (r w) -> c r w", w=WP)
                R = _rows(e)
                if not dense:
                    dvv = act_view(dst_t, C, e)
                tg_end = TGR  # next transpose-group boundary
                for c_i in range(R // 2):
                    rd = 2 * c_i
                    ps = ppool.tile([C, 512], F32, tag="cpsum")
                    if tag == "c6":
                        for t9 in range(9):
                            dy, dx = t9 // 3 - 1, t9 % 3 - 1
                            nc.tensor.matmul(
                                ps[:], wsl[:kw, t9 * C:(t9 + 1) * C],
                                svf[:C, rd + 1 + dy:rd + 3 + dy,
                                    1 + dx:WP - 1 + dx],
                                start=(t9 == 0), stop=(t9 == 8))
                    elif tag == "c7":
                        for j in range(6):
                            dx = j % 3 - 1
                            r0_ = rd if j < 3 else rd + 1
                            nc.tensor.matmul(
                                ps[:], wsl[:kw, j * C:(j + 1) * C],
                                svf[:kw, r0_:r0_ + 2, 1 + dx:WP - 1 + dx],
                                start=(j == 0), stop=(j == 5))
                    else:  # c8, c9: 3-stack, 3 MMs
                        for j in range(3):
                            dx = j - 1
                            nc.tensor.matmul(
                                ps[:], wsl[:kw, j * C:(j + 1) * C],
                                svf[:kw, rd:rd + 2, 1 + dx:WP - 1 + dx],
                                start=(j == 0), stop=(j == 2))
                    if dense:
                        nc.scalar.activation(
                            dst_t[:C, rd * 256:(rd + 2) * 256], ps[:],
                            AF.Relu, bias=cb_t[tag][:C, st:st + 1])
                    else:
                        nc.scalar.activation(
                            dvv[:, rd:rd + 2, 1:WP - 1],
                            ps[:].rearrange("c (a w) -> c a w", w=WW), AF.Relu,
                            bias=cb_t[tag][:C, st:st + 1])
                        chunk_epilogue(dst_t, C, e, rd, 2, dst_stack, qi=c_i)
                    # fused fwd-projection transposes per TGR-row group
                    if fwd is not None and (rd + 2 >= tg_end or rd + 2 >= R):
                        lo, hi = tg_end - TGR, min(tg_end, R)
                        tg_end += TGR
                        olo, ohi = max(lo, fwd.e), min(hi, fwd.e + OWN)
                        fwd.emit_pending()
                        if ohi > olo:
                            n = ohi - olo
                            xt = xtp.tile([128, 2 * TGR * 128], F16, tag="xt")
                            xt3 = xt[:, :2 * n * fwd.Ci].rearrange(
                                "p (k f) -> p k f", f=fwd.Ci)
                            qrot[c_i % 2].dma_start_transpose(
                                xt3, dst_t[:fwd.Ci, olo * 256:ohi * 256])
                            fwd.pending = (xt3, olo, n)
                if fwd is not None:
                    fwd.emit_pending()

            # ---------------- fno block (after pxf is accumulated) ----------
            def fno_block(tag, src_t, dst_t, dst_stack=1, fwd=None):
                Ci, Co, e = FNOD[tag]
                kt = (Ci + 127) // 128
                cp = min(Ci, 128)
                R = _rows(e)
                moff = EMAX - e
                dv = act_view(dst_t, Co, e)
                pxf = fwd.pxf

                # ---- ship batch-masked partial spectrum; AllReduce (8) ----
                s0 = tmp1.tile([cp, kt * 32], F32, tag="xfp0", name="s0")
                s1 = tmp1.tile([cp, kt * 32], F32, tag="xfp1", name="s1")
                nc.scalar.activation(s0[:], pxf[:], AF.Copy,
                                     scale=bsel_t[:cp, 0:1])
                nc.scalar.activation(s1[:], pxf[:], AF.Copy,
                                     scale=bsel_t[:cp, 1:2])
                nc.sync.dma_start(cc_in[tag][0], s0[:])
                nc.scalar.dma_start(cc_in[tag][1], s1[:])
                nc.gpsimd.collective_compute(
                    "AllReduce", mybir.AluOpType.add, replica_groups=ALL8,
                    ins=[cc_in[tag][:]], outs=[cc_out[tag][:]])

                # ---- pass A: 1x1 conv (independent of the collective) ----
                rd = 0
                while rd < R:
                    cr = min(4, R - rd)
                    nh = (cr + 1) // 2
                    ps = fpsp.tile([Co, 1024], F32, tag="fps", name="psA")
                    if tag == "f5":
                        xg = stm2.tile([128, kt * 1024], F16, tag="cwx",
                                       name="xg")
                        for k in range(kt):
                            nc.sync.dma_start(
                                xg[:, k * 1024:k * 1024 + cr * 256],
                                x5b_sl[k * 128:(k + 1) * 128,
                                       rd * WW:(rd + cr) * WW])
                        for h in range(nh):
                            w_ = min(512, cr * 256 - h * 512)
                            for k in range(kt):
                                nc.tensor.matmul(
                                    ps[:, h * 512:h * 512 + w_],
                                    bwT_t[tag][:, k * Co:(k + 1) * Co],
                                    xg[:, k * 1024 + h * 512:
                                       k * 1024 + h * 512 + w_],
                                    start=(k == 0), stop=(k == kt - 1),
                                    skip_group_check=True)
                    else:
                        for h in range(nh):
                            w_ = min(512, cr * 256 - h * 512)
                            nc.tensor.matmul(
                                ps[:, h * 512:h * 512 + w_],
                                bwT_t[tag][:Ci, :Co],
                                src_t[:Ci, rd * 256 + h * 512:
                                      rd * 256 + h * 512 + w_],
                                start=True, stop=True,
                                skip_group_check=True)
                    nc.scalar.activation(
                        dv[:, rd:rd + cr, 1:WP - 1],
                        ps[:, :cr * 256].rearrange("c (a w) -> c a w", w=WW),
                        AF.Copy)
                    rd += cr

                # ---- read back reduced spectrum (own batch); mix ----
                t0 = tmp1.tile([cp, kt * 32], F32, tag="xfp0", name="t0")
                t1 = tmp1.tile([cp, kt * 32], F32, tag="xfp1", name="t1")
                nc.sync.dma_start(t0[:], cc_out[tag][0])
                nc.scalar.dma_start(t1[:], cc_out[tag][1])
                u0 = tmp1.tile([cp, kt * 32], F32, tag="xfr", name="u0")
                nc.scalar.activation(u0[:], t0[:], AF.Copy,
                                     scale=bsel_t[:cp, 0:1])
                u1 = tmp1.tile([cp, kt * 32], F32, tag="xfr1", name="u1")
                nc.scalar.activation(u1[:], t1[:], AF.Copy,
                                     scale=bsel_t[:cp, 1:2])
                xfT = tmp1.tile([cp, kt * 32], F16, tag="xfT")
                nc.vector.tensor_add(xfT[:], u0[:], u1[:])
                xfN = tmp1.tile([cp, kt * 32], F16, tag="xfN")
                xfT3 = xfT[:].rearrange("p (a b) -> p a b", b=2)
                xfN3 = xfN[:].rearrange("p (a b) -> p a b", b=2)
                nc.scalar.mul(xfN3[:, :, 0:1], xfT3[:, :, 1:2], -1.0)
                nc.vector.tensor_copy(xfN3[:, :, 1:2], xfT3[:, :, 0:1])

                pof = pspec.tile([Co, 32], F32, tag="spec", name="pof")
                n_grp = 2 * kt
                for mu in range(16):
                    gi = 0
                    for k in range(kt):
                        off = (k * 16 + mu) * 2 * Co
                        mws = mixw_t[tag][:, off:off + 2 * Co]
                        c0 = k * 32 + 2 * mu
                        nc.tensor.matmul(
                            pof[:, 2 * mu:2 * mu + 2], mws[:cp, :Co],
                            xfT[:, c0:c0 + 2],
                            start=(gi == 0), stop=(gi == n_grp - 1),
                            skip_group_check=True)
                        gi += 1
                        nc.tensor.matmul(
                            pof[:, 2 * mu:2 * mu + 2], mws[:cp, Co:2 * Co],
                            xfN[:, c0:c0 + 2],
                            start=(gi == 0), stop=(gi == n_grp - 1),
                            skip_group_check=True)
                        gi += 1
                of_sb = tmp1.tile([Co, 32], F16, tag="of_sb")
                nc.scalar.mul(of_sb[:], pof[:], 1.0 / 4096.0)
                pofT = pspec.tile([32, 128], F16, tag="spec", name="pofT")
                nc.tensor.transpose(pofT[:, :Co], of_sb[:], ident[:Co, :Co])
                ofb = tmp1.tile([32, 128], F16, tag="ofb")
                nc.vector.tensor_copy(ofb[:, :Co], pofT[:, :Co])

                # ---- pass B: spectral add + gelu + skip (+mask/stack) ----
                ngr = (R + GR - 1) // GR
                ci = 0
                for g in range(ngr):
                    rg = min(GR, R - g * GR)
                    gch = stm2.tile([32, GR * 256], F16, tag="gbch")
                    nc.sync.dma_start(
                        gch[:, :rg * 256],
                        gb[:, (g * GR + moff) * WW:(g * GR + moff + rg) * WW])
                    sk = stm2.tile([Co, GR * 256], F16, tag="skipch")
                    nc.scalar.dma_start(
                        sk[:, :rg * 256],
                        skips[tag][:Co, g * GR * WW:(g * GR + rg) * WW])
                    ro = 0
                    while ro < rg:
                        cr = min(4, rg - ro)
                        rd = g * GR + ro
                        nh = (cr + 1) // 2
                        ps = fpsp.tile([Co, 1024], F32, tag="fps", name="psB")
                        for h in range(nh):
                            w_ = min(512, cr * 256 - h * 512)
                            nc.tensor.matmul(
                                ps[:, h * 512:h * 512 + w_], ofb[:, :Co],
                                gch[:, ro * 256 + h * 512:
                                    ro * 256 + h * 512 + w_],
                                start=True, stop=True, skip_group_check=True)
                        dslice = dv[:, rd:rd + cr, 1:WP - 1]
                        nc.vector.tensor_add(
                            dslice, dslice,
                            ps[:, :cr * 256].rearrange("c (a w) -> c a w",
                                                       w=WW))
                        nc.scalar.activation(dslice, dslice, AF.Gelu,
                                             bias=bb_t[tag][:Co, 0:1])
                        nc.gpsimd.tensor_add(
                            dslice, dslice,
                            sk[:, ro * 256:(ro + cr) * 256]
                            .rearrange("c (a w) -> c a w", w=WW))
                        chunk_epilogue(dst_t, Co, e, rd, cr, dst_stack, qi=ci)
                        ci += 1
                        ro += cr

            # ---------------- f5 forward projection (from x5T) -------------
            def f5_forward(fwd):
                kt = fwd.kt
                nmm = 2 * OWN
                for wt in range(2):
                    for hb in range(OWN // 8):
                        ch = stm3.tile([128, 8 * 256], F16, tag="xtc")
                        nc.sync.dma_start(
                            ch[:].rearrange("w (h c) -> w h c", c=256),
                            x5T[wt, hb * 8:(hb + 1) * 8]
                            .rearrange("h w c -> w h c"))
                        for h in range(8):
                            for k in range(kt):
                                nc.tensor.matmul(
                                    fwd.pxf[:, k * 32:(k + 1) * 32],
                                    ch[:, h * 256 + k * 128:
                                       h * 256 + (k + 1) * 128],
                                    fb_sl(wt, hb * 8 + h),
                                    start=(fwd.i[k] == 0),
                                    stop=(fwd.i[k] == nmm - 1),
                                    skip_group_check=True)
                                fwd.i[k] += 1

            # ---------------- the network ----------------
            fw5 = FwdState("f5")
            f5_forward(fw5)
            x5u = new_act("A", 9)
            fno_block("f5", None, x5u, dst_stack=1, fwd=fw5)
            x6a = new_act("B", 8)
            conv_stage("c6", 128, 0, 8, x5u, 9, x6a)
            x6b = new_act("A", 7)
            conv_stage("c6", 128, 1, 7, x6a, 8, x6b)
            x6 = new_dense("B", 128, _rows(6))
            fw6 = FwdState("f6")
            conv_stage("c6", 128, 2, 6, x6b, 7, x6, dense=True, fwd=fw6)
            x6u = new_act("A", 6)
            fno_block("f6", x6, x6u, dst_stack=2, fwd=fw6)
            x7a = new_act("B", 5)
            conv_stage("c7", 64, 0, 5, x6u, 6, x7a, dst_stack=2)
            x7b = new_act("A", 4)
            conv_stage("c7", 64, 1, 4, x7a, 5, x7b, dst_stack=2)
            x7 = new_dense("B", 64, _rows(3))
            fw7 = FwdState("f7")
            conv_stage("c7", 64, 2, 3, x7b, 4, x7, dense=True, fwd=fw7)
            x7u = new_act("A", 3)
            fno_block("f7", x7, x7u, dst_stack=3, fwd=fw7)
            x8a = new_act("B", 2)
            conv_stage("c8", 32, 0, 2, x7u, 3, x8a, dst_stack=3)
            x8 = new_dense("A", 32, _rows(1))
            fw8 = FwdState("f8")
            conv_stage("c8", 32, 1, 1, x8a, 2, x8, dense=True, fwd=fw8)
            x8u = new_act("B", 1)
            fno_block("f8", x8, x8u, dst_stack=3, fwd=fw8)
            x9 = new_dense("A", 16, OWN)
            conv_stage("c9", 16, 0, 0, x8u, 1, x9, dense=True)

            # final 1x1 conv (16 -> 2), owned rows only; 4-row chunks
            for g in range(OWN // 4):
                rd = 4 * g
                ps = fpsp.tile([2, 1024], F32, tag="fps", name="psO")
                for h in range(2):
                    nc.tensor.matmul(ps[:, h * 512:(h + 1) * 512], ow_t[:],
                                     x9[:16, rd * 256 + h * 512:
                                        rd * 256 + (h + 1) * 512],
                                     start=True, stop=True,
                                     skip_group_check=True)
                oc = stm2.tile([2, 1024], F32, tag="outch", name="outch")
                nc.scalar.activation(oc[:], ps[:], AF.Identity, bias=ob_t[:])
                nc.scalar.dma_start(out_sl[:, rd * WW:(rd + 4) * WW], oc[:])

    nc.compile()
    return nc


# ---------------------------------------------------------------------------
# host side
# ---------------------------------------------------------------------------

def _slice_rows(x, lo, hi):
    """x: (C, 256, 256) -> (C, hi-lo, 256) zero-padded out of range."""
    C = x.shape[0]
    out = np.zeros((C, hi - lo, WW), np.float32)
    a, b2 = max(lo, 0), min(hi, HH)
    if b2 > a:
        out[:, a - lo:b2 - lo] = x[:, a:b2]
    return out


def _host_inputs(inputs):
    i = inputs
    maps = []
    kk, ll = np.meshgrid(np.arange(M), np.arange(M), indexing="ij")
    kf = kk.reshape(-1).astype(np.float64)   # mu = 4k + l
    lf = ll.reshape(-1).astype(np.float64)
    alpha32 = np.where(lf == 0, 1.0, 2.0).repeat(2)  # per 32-comp row

    def basis(rows_abs, wvals):  # -> (32, len(rows), len(w))
        th = 2 * np.pi * (kf[:, None, None] * rows_abs[None, :, None] / HH
                          + lf[:, None, None] * wvals[None, None, :] / WW)
        out = np.empty((32, len(rows_abs), len(wvals)), np.float32)
        out[0::2] = np.cos(th) / 256.0
        out[1::2] = -np.sin(th) / 256.0
        return out

    # weights (identical on all cores)
    cw_np = {}
    cb_np = {}
    for tag, C, nst, _ in CONVS:
        w = np.asarray(i[tag + "_w"], np.float32)     # (n, co, ci, 3, 3)
        wt = w.transpose(0, 3, 4, 2, 1)               # (n, dy, dx, ci, co)
        if tag == "c6":
            cw_np[tag] = np.ascontiguousarray(wt.reshape(nst, 9, C, C))
        elif tag == "c7":
            cwv = np.zeros((nst, 6, 128, C), np.float32)
            for dx in range(3):
                cwv[:, dx, :C] = wt[:, 0, dx]         # dy=-1 via h0
                cwv[:, dx, C:2 * C] = wt[:, 1, dx]    # dy=0 via h1
                cwv[:, 3 + dx, C:2 * C] = wt[:, 2, dx]  # dy=+1 via h1
            cw_np[tag] = cwv
        else:  # c8, c9: 3-stack
            cwv = np.zeros((nst, 3, 3 * C, C), np.float32)
            for dx in range(3):
                for dy in range(3):
                    cwv[:, dx, dy * C:(dy + 1) * C] = wt[:, dy, dx]
            cw_np[tag] = cwv
        cw_np[tag] = cw_np[tag].astype(np.float16)
        cbv = np.zeros((128, nst), np.float32)
        cbv[:C] = np.asarray(i[tag + "_b"], np.float32).T
        cb_np[tag] = cbv
    fno_np = {}
    for tag, Ci, Co, e in FNOS:
        kt = (Ci + 127) // 128
        bw = np.asarray(i[tag + "_bw"], np.float32)[:, :, 0, 0]  # (oc, ic)
        full = np.ascontiguousarray(bw.T)                        # (ic, oc)
        bwT_ = np.zeros((128, kt * Co), np.float32)
        for k in range(kt):
            w = min(128, Ci - k * 128)
            bwT_[:w, k * Co:(k + 1) * Co] = full[k * 128:k * 128 + w]
        wr = np.asarray(i[tag + "_wr"], np.float32)   # (ic, oc, 4, 4)
        wi = np.asarray(i[tag + "_wi"], np.float32)
        # mixw layout: (128, kt*16*2*Co): slab for (k, mu) at (k*16+mu)*2*Co,
        # first Co cols = wr rows k*128.., next Co = wi rows
        mw = np.zeros((128, kt * 16 * 2 * Co), np.float32)
        for k in range(kt):
            w = min(128, Ci - k * 128)
            for mu in range(16):
                kk_, ll_ = mu // 4, mu % 4
                off = (k * 16 + mu) * 2 * Co
                mw[:w, off:off + Co] = wr[k * 128:k * 128 + w, :, kk_, ll_]
                mw[:w, off + Co:off + 2 * Co] = wi[k * 128:k * 128 + w, :, kk_, ll_]
        bbv = np.zeros((128, 1), np.float32)
        bbv[:Co, 0] = np.asarray(i[tag + "_bb"], np.float32)
        fno_np[tag] = (bwT_.astype(np.float16), bbv,
                       (mw * 4096.0).astype(np.float16))
    owT_np = np.ascontiguousarray(
        np.asarray(i["ow"], np.float32)[:, :, 0, 0].T).astype(np.float16)
    ob_np = np.asarray(i["ob"], np.float32)[:, None]

    skips_full = {"f5": np.asarray(i["x4"], np.float32),
                  "f6": np.asarray(i["x3"], np.float32),
                  "f7": np.asarray(i["x2"], np.float32),
                  "f8": np.asarray(i["x1"], np.float32)}
    x5 = np.asarray(i["x5"], np.float32)
    wvals = np.arange(WW, dtype=np.float64)
    K_rows9 = _rows(9)

    for core in range(NCORES):
        b, q = divmod(core, 4)
        r0 = OWN * q
        m = {}
        m["x5b_sl"] = _slice_rows(x5[b], r0 - 9, r0 + OWN + 9).reshape(256, -1).astype(np.float16)
        m["x4_sl"] = _slice_rows(skips_full["f5"][b], r0 - 9, r0 + OWN + 9).reshape(128, -1).astype(np.float16)
        m["x3_sl"] = _slice_rows(skips_full["f6"][b], r0 - 6, r0 + OWN + 6).reshape(64, -1).astype(np.float16)
        m["x2_sl"] = _slice_rows(skips_full["f7"][b], r0 - 3, r0 + OWN + 3).reshape(32, -1).astype(np.float16)
        m["x1_sl"] = _slice_rows(skips_full["f8"][b], r0 - 1, r0 + OWN + 1).reshape(16, -1).astype(np.float16)
        xo = x5[b][:, r0:r0 + OWN, :]                       # (256c, 64h, 256w)
        x5T_ = xo.transpose(1, 2, 0).reshape(OWN, 2, 128, 256).transpose(1, 0, 2, 3)
        m["x5T"] = np.ascontiguousarray(x5T_).astype(np.float16)
        # fwd basis (w, (wt h m)) at abs rows r0+h, col wt*128+w
        fbb = basis(np.arange(r0, r0 + OWN, dtype=np.float64), wvals)  # (32,64,256)
        fbb = (fbb.transpose(2, 1, 0)                       # (w256, h, m)
               .reshape(2, 128, OWN, 32)                    # (wt, w, h, m)
               .transpose(1, 0, 2, 3)                       # (w, wt, h, m)
               .reshape(128, 2 * OWN * 32))
        m["fb"] = np.ascontiguousarray(fbb).astype(np.float16)
        rows = np.arange(r0 - 9, r0 + OWN + 9, dtype=np.float64)
        gbb = basis(rows, wvals) * alpha32[:, None, None]
        gbb[:, (rows < 0) | (rows >= HH), :] = 0.0
        m["gb"] = gbb.reshape(32, -1).astype(np.float16)
        mrow = ((rows >= 0) & (rows < HH)).astype(np.float32)
        m["mask_pp"] = np.tile(mrow[None, :], (128, 1)).astype(np.float16)
        bs = np.zeros((128, 2), np.float32)
        bs[:, b] = 1.0
        m["bsel"] = bs
        m["zpadb"] = np.zeros((128, 2 * K_rows9), np.float16)
        for tag, C, nst, _ in CONVS:
            m[tag + "w"] = cw_np[tag]
            m[tag + "b"] = cb_np[tag]
        for tag, Ci, Co, e in FNOS:
            bwT_, bb_, mw_ = fno_np[tag]
            m[tag + "_bwT"] = bwT_
            m[tag + "_bb"] = bb_
            m[tag + "_mixw"] = mw_
        m["owT"] = owT_np
        m["ob"] = ob_np
        maps.append(m)
    return maps


_NC_CACHE = {}


def kernel(**inputs):
    if "nc" not in _NC_CACHE:
        _NC_CACHE["nc"] = _build_nc()
    nc = _NC_CACHE["nc"]
    maps = _host_inputs(inputs)
    res = run_bass_kernel_spmd(nc, maps, list(range(NCORES)), trace=False)
    out = np.zeros((B, 2, HH, WW), np.float32)
    for core in range(NCORES):
        b, q = divmod(core, 4)
        r0 = OWN * q
        out[b, :, r0:r0 + OWN, :] = res.results[core]["out_sl"].reshape(2, OWN, WW)
    return out


# revision 17
# speedup vs baseline: 1.0311x; 1.0311x over previous
"""Trainium2 Bass kernel for nn_Decoder_34325378630277 (FNO-UNet decoder).

Sharding: 8 cores = 2 batches x 4 row-quarters (64 owned rows each).
Conv halos handled by extended recompute (host supplies zero-padded row
slices).

v2 design:
- FNO forward spectral projection is fused into the conv stage that
  produces the FNO source: the source is stored DENSE (c, r*256), and
  per 8-row group an SBUF->SBUF XBAR dma-transpose produces (w, blk, c)
  slabs that feed accumulating matmuls into a per-channel spectrum
  pxfT (Ci, 32) held in PSUM across the stage.  This removes the
  DRAM-bounce transposes (which were both slow and racy).
- The spectrum AllReduce fires right at conv end, hidden behind the
  1x1-conv pass A; the channel mix happens in (Ci, m) orientation so no
  spectrum transposes are needed.
- Pass A/B use 4-row chunks (2-bank PSUM tiles), with the gelu on the
  scalar engine, the spectral add on vector, and the skip add on the
  otherwise-idle Pool (gpsimd) engine.
- Everything bf16 except PSUM accumulation and the final output.
"""
import sys, types
import os
STACK_ENGINE = os.environ.get('STACK_ENGINE', '0') == '1'

sys.path.insert(0, "/opt/trn_rl_repo")
import numpy as np
import ml_dtypes

# NTFF profile hook shim (lets trace=True work under axon; harmless otherwise)
try:
    import antenv  # noqa: F401
    if "antenv.axon_hooks" not in sys.modules:
        _h = {"hook": None}
        _m = types.ModuleType("antenv.axon_hooks")
        _m.set_axon_ntff_profile_hook = lambda h: _h.__setitem__("hook", h)
        _m.get_axon_ntff_profile_hook = lambda: _h["hook"]
        sys.modules["antenv.axon_hooks"] = _m
        from trn_agent_boot.trn_boot import _ntff_profile_via_ctypes
        _m.set_axon_ntff_profile_hook(
            _ntff_profile_via_ctypes("/opt/axon/libaxon_pjrt.so"))
except Exception:
    pass

import concourse.bass as bass
import concourse.bacc as bacc
import concourse.tile as tile
from concourse import mybir, masks
from concourse.bass_utils import run_bass_kernel_spmd

F32 = mybir.dt.float32
F16 = mybir.dt.float16
AF = mybir.ActivationFunctionType

B, HH, WW, NF = 2, 256, 256, 16
OWN = 64
NCORES = 8
WP = WW + 2          # padded width (zero cols at 0 and WP-1)
EMAX = 9             # x5u halo
M = 4                # modes kept per axis
GR = 8               # row-group granularity for streaming loads
TGR = 8              # transpose group rows (fused fwd projection)

# fno stages: (tag, Ci, Co, e)
FNOS = [("f5", 256, 128, 9), ("f6", 128, 64, 6), ("f7", 64, 32, 3), ("f8", 32, 16, 1)]
FNOD = {t: (ci, co, e) for t, ci, co, e in FNOS}
# conv blocks: (tag, C, n_stages, e_dst per stage)
CONVS = [("c6", 128, 3, [8, 7, 6]), ("c7", 64, 3, [5, 4, 3]),
         ("c8", 32, 2, [2, 1]), ("c9", 16, 1, [0])]
CWSHP = {"c6": (3, 9, 128, 128), "c7": (3, 6, 128, 64),
         "c8": (2, 3, 96, 32), "c9": (1, 3, 48, 16)}


def _rows(e):
    return OWN + 2 * e


# ---------------------------------------------------------------------------
# device program
# ---------------------------------------------------------------------------

def _build_nc():
    nc = bacc.Bacc("TRN2", target_bir_lowering=False, debug=False,
                   num_devices=NCORES)

    def din(name, shape, dt):
        return nc.dram_tensor(name, list(shape), dt, kind="ExternalInput").ap()

    # --- external inputs (per-core data) ---
    x5b_sl = din("x5b_sl", (256, _rows(9) * WW), F16)
    x4_sl = din("x4_sl", (128, _rows(9) * WW), F16)
    x3_sl = din("x3_sl", (64, _rows(6) * WW), F16)
    x2_sl = din("x2_sl", (32, _rows(3) * WW), F16)
    x1_sl = din("x1_sl", (16, _rows(1) * WW), F16)
    skips = {"f5": x4_sl, "f6": x3_sl, "f7": x2_sl, "f8": x1_sl}
    x5T = din("x5T", (2, OWN // 8, 128, 8 * 256), F16)  # (wt, hb, w, (h c))
    zpadb = din("zpadb", (128, 2 * _rows(9)), F16)     # zeros for pad cols
    fb_in = din("fb", (128, 2 * OWN * 32), F16)        # (w, (wt h m)) fwd basis
    gb = din("gb", (32, _rows(9) * WW), F16)           # inv basis rows r0-9..r0+73
    mask_pp = din("mask_pp", (128, _rows(9)), F16)     # in-image mask, replicated
    bsel = din("bsel", (128, 2), F32)                  # one-hot batch select
    cw = {}
    cb = {}
    for tag, C, nst, _ in CONVS:
        # host pre-arranges to (stage, i, (t o)) so the load is contiguous
        nst_, nt_, kw_, C_ = CWSHP[tag]
        cw[tag] = din(tag + "w", (nst_, 128, nt_ * C_), F16)
        cb[tag] = din(tag + "b", (128, nst), F32)
    bwT = {}
    bb = {}
    mixw = {}
    for tag, Ci, Co, _ in FNOS:
        kt = (Ci + 127) // 128
        bwT[tag] = din(tag + "_bwT", (128, kt * Co), F16)
        bb[tag] = din(tag + "_bb", (128, 1), F32)
        mixw[tag] = din(tag + "_mixw", (128, kt * 16 * 2 * Co), F16)
    owT = din("owT", (16, 2), F16)
    ob = din("ob", (2, 1), F32)

    out_sl = nc.dram_tensor("out_sl", [2, OWN * WW], F32,
                            kind="ExternalOutput").ap()

    # --- internal dram (collective buffers) ---
    cc_in = {}
    cc_out = {}
    for tag, Ci, Co, e in FNOS:
        kt = (Ci + 127) // 128
        cp = min(Ci, 128)
        cc_in[tag] = nc.dram_tensor("ccin_" + tag, [2, cp, kt * 32], F32).ap()
        cc_out[tag] = nc.dram_tensor("ccout_" + tag, [2, cp, kt * 32], F32,
                                     addr_space="Shared").ap()
    ccw_in = nc.dram_tensor("ccw_in", [1, 8], F32).ap()
    ccw_out = nc.dram_tensor("ccw_out", [1, 8], F32, addr_space="Shared").ap()

    with tile.TileContext(nc) as tc:
        import contextlib
        est = contextlib.ExitStack()
        with est:
            big = est.enter_context(tc.tile_pool(name="big", bufs=1))
            cst = est.enter_context(tc.tile_pool(name="cst", bufs=1))
            stm2 = est.enter_context(tc.tile_pool(name="stm2", bufs=2))
            stm3 = est.enter_context(tc.tile_pool(name="stm3", bufs=3))
            xtp = est.enter_context(tc.tile_pool(name="xtp", bufs=2))
            tmp1 = est.enter_context(tc.tile_pool(name="tmp1", bufs=1))
            ppool = est.enter_context(
                tc.tile_pool(name="ppool", bufs=3, space="PSUM"))
            fpsp = est.enter_context(
                tc.tile_pool(name="fpsp", bufs=2, space="PSUM"))
            pspec = est.enter_context(
                tc.tile_pool(name="pspec", bufs=1, space="PSUM"))

            ALL8 = [list(range(NCORES))]
            # warmup collective (absorbs communicator init early)
            nc.gpsimd.collective_compute(
                "AllReduce", mybir.AluOpType.add, replica_groups=ALL8,
                ins=[ccw_in[:]], outs=[ccw_out[:]])

            # --- constants resident in SBUF (split across the two HWDGE qs) ---
            ident = cst.tile([128, 128], F16, tag="ident")
            masks.make_identity(nc, ident[:])
            fb_t = cst.tile([128, 2 * OWN * 32], F16, tag="fb")
            nc.sync.dma_start(fb_t[:], fb_in[:])
            mask_t = cst.tile([128, _rows(9)], F16, tag="mask")
            nc.scalar.dma_start(mask_t[:], mask_pp[:])
            bsel_t = cst.tile([128, 2], F32, tag="bsel")
            nc.scalar.dma_start(bsel_t[:], bsel[:])

            def fb_sl(wt, h):  # (128, 32) lhsT slab for fwd basis
                o = (wt * OWN + h) * 32
                return fb_t[:, o:o + 32]

            bwT_t = {}
            bb_t = {}
            mixw_t = {}
            for i, (tag, Ci, Co, e) in enumerate(FNOS):
                kt = (Ci + 127) // 128
                q = nc.sync if tag == "f5" else nc.scalar
                bwT_t[tag] = cst.tile([128, kt * Co], F16, tag="bw" + tag,
                                      name="bw_" + tag)
                q.dma_start(bwT_t[tag][:], bwT[tag][:])
                bb_t[tag] = cst.tile([128, 1], F32, tag="bb" + tag,
                                     name="bbt_" + tag)
                q.dma_start(bb_t[tag][:], bb[tag][:])
            cb_t = {}
            cw_t = {}
            for i, (tag, C, nst, _) in enumerate(CONVS):
                cb_t[tag] = cst.tile([128, nst], F32, tag="cb" + tag,
                                     name="cbt_" + tag)
                nc.scalar.dma_start(cb_t[tag][:], cb[tag][:])
            ow_t = cst.tile([16, 2], F16, tag="ow")
            nc.scalar.dma_start(ow_t[:], owT[:])
            ob_t = cst.tile([2, 1], F32, tag="ob")
            nc.scalar.dma_start(ob_t[:], ob[:])

            def load_late_consts():
                # big constant loads deferred so f5's streams go first
                for i, (tag, Ci, Co, e) in enumerate(FNOS):
                    kt = (Ci + 127) // 128
                    mixw_t[tag] = cst.tile([128, kt * 16 * 2 * Co], F16,
                                           tag="mw" + tag, name="mwt_" + tag)
                    nc.scalar.dma_start(mixw_t[tag][:], mixw[tag][:])
                for i, (tag, C, nst, _) in enumerate(CONVS):
                    nt, kw = CWSHP[tag][1], CWSHP[tag][2]
                    cw_t[tag] = []
                    for st in range(nst):
                        w_t = cst.tile([128, nt * C], F16, tag=f"cw{tag}{st}",
                                       name=f"cwt_{tag}{st}")
                        qrot[st % 2].dma_start(w_t[:], cw[tag][st])
                        cw_t[tag].append(w_t)

            # --- big activation slots (bf16) ---
            def new_act(slot, e):
                """Padded activation (128, R*WP) with zeroed pad columns."""
                t = big.tile([128, _rows(e) * WP], F16, tag=slot,
                             name="act_" + slot)
                R = _rows(e)
                z = t[:].rearrange("c (r w) -> c r w", w=WP)
                nc.scalar.dma_start(z[:, :, 0:1], zpadb[:, :R])
                nc.scalar.dma_start(z[:, :, WP - 1:WP], zpadb[:, R:2 * R])
                return t

            def new_dense(slot, C, R):
                """Dense activation (C, R*256), no pads (FNO sources, x9)."""
                return big.tile([128, R * 256], F16, tag=slot,
                                name="dact_" + slot)

            def act_view(t, C, e):
                return t[:C].rearrange("c (r w) -> c r w", w=WP)

            # per-chunk epilogue for padded dsts: boundary mask + stack copies
            qrot = [nc.sync, nc.scalar]

            # stack copies on compute engines (vector/pool) — the sbuf->sbuf
            # DMA route showed row-granular races under load
            erot = [nc.vector, nc.gpsimd]

            def chunk_epilogue(dst_t, C, e, rd, cr, nstack, qi=0):
                R = _rows(e)
                moff = EMAX - e
                dv = act_view(dst_t, C, e)
                if rd < 9 or rd + cr > R - 9:
                    nc.vector.tensor_mul(
                        dv[:, rd:rd + cr, :], dv[:, rd:rd + cr, :],
                        mask_t[:C, moff + rd:moff + rd + cr]
                        .broadcast_to((C, cr, WP)))
                if nstack > 1:
                    v = dst_t[:].rearrange("c (r w) -> c r w", w=WP)
                    for k in range(1, nstack):
                        lo = max(0, rd - k)
                        hi = min(R - k, rd + cr - k)
                        if hi > lo:
                            if STACK_ENGINE:
                                erot[(qi + k) % 2].tensor_copy(
                                    v[k * C:(k + 1) * C, lo:hi, :],
                                    v[0:C, lo + k:hi + k, :])
                            else:
                                qrot[(qi + k) % 2].dma_start(
                                    v[k * C:(k + 1) * C, lo:hi, :],
                                    v[0:C, lo + k:hi + k, :])

            # ---------- fused forward-projection state ----------
            class FwdState:
                def __init__(self, tag):
                    Ci, Co, e = FNOD[tag]
                    self.tag, self.Ci, self.e = tag, Ci, e
                    self.kt = (Ci + 127) // 128
                    cp = min(Ci, 128)
                    self.pxf = pspec.tile([cp, self.kt * 32], F32,
                                          tag="spec", name="pxf_" + tag)
                    self.i = [0] * self.kt      # per-chain matmul counter
                    self.n = 2 * OWN            # matmuls per chain
                    self.pending = None

                def emit_pending(self):
                    if self.pending is None:
                        return
                    xt3, olo, nrows = self.pending
                    self.pending = None
                    for j in range(nrows):
                        h = olo - self.e + j
                        for wt in range(2):
                            k = 0  # kt==1 for fused tags (f6/f7/f8)
                            nc.tensor.matmul(
                                self.pxf[:, 0:32],
                                xt3[:, 2 * j + wt, :],
                                fb_sl(wt, h),
                                start=(self.i[k] == 0),
                                stop=(self.i[k] == self.n - 1),
                                skip_group_check=True)
                            self.i[k] += 1

            # ---------------- conv stage ----------------
            def conv_stage(tag, C, st, e, src_t, src_e, dst_t,
                           dst_stack=1, dense=False, fwd=None):
                nt, kw = CWSHP[tag][1], CWSHP[tag][2]
                wsl = cw_t[tag][st]
                svf = src_t[:].rearrange("c (r w) -> c r w", w=WP)
                R = _rows(e)
                if not dense:
                    dvv = act_view(dst_t, C, e)
                if fwd is not None:
                    # owned rows first so the spectrum (and its AllReduce)
                    # completes early; halo chunks after hide the collective
                    elo = fwd.e & ~1
                    ehi = (fwd.e + OWN + 1) & ~1
                    order = (list(range(elo // 2, ehi // 2))
                             + list(range(0, elo // 2))
                             + list(range(ehi // 2, R // 2)))
                    tg_end = fwd.e + TGR  # owned-row transpose boundaries
                else:
                    order = list(range(R // 2))
                for c_i in order:
                    rd = 2 * c_i
                    ps = ppool.tile([C, 512], F32, tag="cpsum")
                    if tag == "c6":
                        for t9 in range(9):
                            dy, dx = t9 // 3 - 1, t9 % 3 - 1
                            nc.tensor.matmul(
                                ps[:], wsl[:kw, t9 * C:(t9 + 1) * C],
                                svf[:C, rd + 1 + dy:rd + 3 + dy,
                                    1 + dx:WP - 1 + dx],
                                start=(t9 == 0), stop=(t9 == 8))
                    elif tag == "c7":
                        for j in range(6):
                            dx = j % 3 - 1
                            r0_ = rd if j < 3 else rd + 1
                            nc.tensor.matmul(
                                ps[:], wsl[:kw, j * C:(j + 1) * C],
                                svf[:kw, r0_:r0_ + 2, 1 + dx:WP - 1 + dx],
                                start=(j == 0), stop=(j == 5))
                    else:  # c8, c9: 3-stack, 3 MMs
                        for j in range(3):
                            dx = j - 1
                            nc.tensor.matmul(
                                ps[:], wsl[:kw, j * C:(j + 1) * C],
                                svf[:kw, rd:rd + 2, 1 + dx:WP - 1 + dx],
                                start=(j == 0), stop=(j == 2))
                    if dense:
                        nc.scalar.activation(
                            dst_t[:C, rd * 256:(rd + 2) * 256], ps[:],
                            AF.Relu, bias=cb_t[tag][:C, st:st + 1])
                    else:
                        nc.scalar.activation(
                            dvv[:, rd:rd + 2, 1:WP - 1],
                            ps[:].rearrange("c (a w) -> c a w", w=WW), AF.Relu,
                            bias=cb_t[tag][:C, st:st + 1])
                        chunk_epilogue(dst_t, C, e, rd, 2, dst_stack, qi=c_i)
                    # fused fwd-projection transposes per TGR owned rows
                    if fwd is not None and rd + 2 >= tg_end \
                            and tg_end <= fwd.e + OWN:
                        olo, ohi = tg_end - TGR, tg_end
                        tg_end += TGR
                        fwd.emit_pending()
                        n = ohi - olo
                        xt = xtp.tile([128, 2 * TGR * 128], F16, tag="xt")
                        xt3 = xt[:, :2 * n * fwd.Ci].rearrange(
                            "p (k f) -> p k f", f=fwd.Ci)
                        qrot[c_i % 2].dma_start_transpose(
                            xt3, dst_t[:fwd.Ci, olo * 256:ohi * 256])
                        fwd.pending = (xt3, olo, n)
                        if tg_end > fwd.e + OWN:
                            fwd.emit_pending()  # last group: emit now

            # ---------------- fno block (after pxf is accumulated) ----------
            def fno_block(tag, src_t, dst_t, dst_stack=1, fwd=None):
                Ci, Co, e = FNOD[tag]
                kt = (Ci + 127) // 128
                cp = min(Ci, 128)
                R = _rows(e)
                moff = EMAX - e
                dv = act_view(dst_t, Co, e)
                pxf = fwd.pxf

                # ---- ship batch-masked partial spectrum; AllReduce (8) ----
                s0 = tmp1.tile([cp, kt * 32], F32, tag="xfp0", name="s0")
                s1 = tmp1.tile([cp, kt * 32], F32, tag="xfp1", name="s1")
                nc.scalar.activation(s0[:], pxf[:], AF.Copy,
                                     scale=bsel_t[:cp, 0:1])
                nc.scalar.activation(s1[:], pxf[:], AF.Copy,
                                     scale=bsel_t[:cp, 1:2])
                nc.sync.dma_start(cc_in[tag][0], s0[:])
                nc.scalar.dma_start(cc_in[tag][1], s1[:])
                nc.gpsimd.collective_compute(
                    "AllReduce", mybir.AluOpType.add, replica_groups=ALL8,
                    ins=[cc_in[tag][:]], outs=[cc_out[tag][:]])

                # ---- pass A (f5 only): 1x1 conv hides the collective ----
                if tag == "f5":
                    rd = 0
                    pi = 0
                    while rd < R:
                        cr = min(4, R - rd)
                        nh = (cr + 1) // 2
                        ps = fpsp.tile([Co, 1024], F32, tag="fps", name="psA")
                        xg = stm2.tile([128, kt * 1024], F16, tag="cwx",
                                       name="xg")
                        for k in range(kt):
                            nc.sync.dma_start(
                                xg[:, k * 1024:k * 1024 + cr * 256],
                                x5b_sl[k * 128:(k + 1) * 128,
                                       rd * WW:(rd + cr) * WW])
                        for h in range(nh):
                            w_ = min(512, cr * 256 - h * 512)
                            for k in range(kt):
                                nc.tensor.matmul(
                                    ps[:, h * 512:h * 512 + w_],
                                    bwT_t[tag][:, k * Co:(k + 1) * Co],
                                    xg[:, k * 1024 + h * 512:
                                       k * 1024 + h * 512 + w_],
                                    start=(k == 0), stop=(k == kt - 1),
                                    skip_group_check=True)
                        eng = nc.scalar if pi % 2 == 0 else nc.vector
                        if pi % 2 == 0:
                            nc.scalar.activation(
                                dv[:, rd:rd + cr, 1:WP - 1],
                                ps[:, :cr * 256]
                                .rearrange("c (a w) -> c a w", w=WW),
                                AF.Copy)
                        else:
                            nc.vector.tensor_copy(
                                dv[:, rd:rd + cr, 1:WP - 1],
                                ps[:, :cr * 256]
                                .rearrange("c (a w) -> c a w", w=WW))
                        pi += 1
                        rd += cr

                # ---- read back reduced spectrum (own batch); mix ----
                t0 = tmp1.tile([cp, kt * 32], F32, tag="xfp0", name="t0")
                t1 = tmp1.tile([cp, kt * 32], F32, tag="xfp1", name="t1")
                nc.sync.dma_start(t0[:], cc_out[tag][0])
                nc.scalar.dma_start(t1[:], cc_out[tag][1])
                u0 = tmp1.tile([cp, kt * 32], F32, tag="xfr", name="u0")
                nc.scalar.activation(u0[:], t0[:], AF.Copy,
                                     scale=bsel_t[:cp, 0:1])
                u1 = tmp1.tile([cp, kt * 32], F32, tag="xfr1", name="u1")
                nc.scalar.activation(u1[:], t1[:], AF.Copy,
                                     scale=bsel_t[:cp, 1:2])
                xfT = tmp1.tile([cp, kt * 32], F16, tag="xfT")
                nc.vector.tensor_add(xfT[:], u0[:], u1[:])
                xfN = tmp1.tile([cp, kt * 32], F16, tag="xfN")
                xfT3 = xfT[:].rearrange("p (a b) -> p a b", b=2)
                xfN3 = xfN[:].rearrange("p (a b) -> p a b", b=2)
                nc.scalar.mul(xfN3[:, :, 0:1], xfT3[:, :, 1:2], -1.0)
                nc.vector.tensor_copy(xfN3[:, :, 1:2], xfT3[:, :, 0:1])

                pof = pspec.tile([Co, 32], F32, tag="spec", name="pof")
                n_grp = 2 * kt
                for mu in range(16):
                    gi = 0
                    for k in range(kt):
                        off = (k * 16 + mu) * 2 * Co
                        mws = mixw_t[tag][:, off:off + 2 * Co]
                        c0 = k * 32 + 2 * mu
                        nc.tensor.matmul(
                            pof[:, 2 * mu:2 * mu + 2], mws[:cp, :Co],
                            xfT[:, c0:c0 + 2],
                            start=(gi == 0), stop=(gi == n_grp - 1),
                            skip_group_check=True)
                        gi += 1
                        nc.tensor.matmul(
                            pof[:, 2 * mu:2 * mu + 2], mws[:cp, Co:2 * Co],
                            xfN[:, c0:c0 + 2],
                            start=(gi == 0), stop=(gi == n_grp - 1),
                            skip_group_check=True)
                        gi += 1
                of_sb = tmp1.tile([Co, 32], F16, tag="of_sb")
                nc.scalar.mul(of_sb[:], pof[:], 1.0 / 4096.0)
                pofT = pspec.tile([32, 128], F16, tag="spec", name="pofT")
                nc.tensor.transpose(pofT[:, :Co], of_sb[:], ident[:Co, :Co])
                ofb = tmp1.tile([32, 128], F16, tag="ofb")
                nc.vector.tensor_copy(ofb[:, :Co], pofT[:, :Co])

                # ---- pass B: spectral add + gelu + skip (+mask/stack) ----
                ngr = (R + GR - 1) // GR
                ci = 0
                for g in range(ngr):
                    rg = min(GR, R - g * GR)
                    gch = stm2.tile([32, GR * 256], F16, tag="gbch")
                    nc.sync.dma_start(
                        gch[:, :rg * 256],
                        gb[:, (g * GR + moff) * WW:(g * GR + moff + rg) * WW])
                    sk = stm2.tile([Co, GR * 256], F16, tag="skipch")
                    nc.scalar.dma_start(
                        sk[:, :rg * 256],
                        skips[tag][:Co, g * GR * WW:(g * GR + rg) * WW])
                    ro = 0
                    while ro < rg:
                        cr = min(4, rg - ro)
                        rd = g * GR + ro
                        nh = (cr + 1) // 2
                        ps = fpsp.tile([Co, 1024], F32, tag="fps", name="psB")
                        for h in range(nh):
                            w_ = min(512, cr * 256 - h * 512)
                            if tag != "f5":
                                # fold the 1x1 conv in (src resident dense)
                                nc.tensor.matmul(
                                    ps[:, h * 512:h * 512 + w_],
                                    bwT_t[tag][:Ci, :Co],
                                    src_t[:Ci, rd * 256 + h * 512:
                                          rd * 256 + h * 512 + w_],
                                    start=True, stop=False,
                                    skip_group_check=True)
                            nc.tensor.matmul(
                                ps[:, h * 512:h * 512 + w_], ofb[:, :Co],
                                gch[:, ro * 256 + h * 512:
                                    ro * 256 + h * 512 + w_],
                                start=(tag == "f5"), stop=True,
                                skip_group_check=True)
                        dslice = dv[:, rd:rd + cr, 1:WP - 1]
                        psv = ps[:, :cr * 256].rearrange("c (a w) -> c a w",
                                                         w=WW)
                        skv = (sk[:, ro * 256:(ro + cr) * 256]
                               .rearrange("c (a w) -> c a w", w=WW))
                        if tag == "f5":
                            # dv holds pass A; RMW: +spec, gelu, +skip
                            nc.vector.tensor_add(dslice, dslice, psv)
                            nc.scalar.activation(dslice, dslice, AF.Gelu,
                                                 bias=bb_t[tag][:Co, 0:1])
                            h2 = cr // 2
                            nc.gpsimd.tensor_add(
                                dslice[:, :h2], dslice[:, :h2], skv[:, :h2])
                            nc.vector.tensor_add(
                                dslice[:, h2:], dslice[:, h2:], skv[:, h2:])
                        else:
                            # psum already has conv+spec: gelu writes dv
                            nc.scalar.activation(dslice, psv, AF.Gelu,
                                                 bias=bb_t[tag][:Co, 0:1])
                            h2 = cr // 2
                            nc.gpsimd.tensor_add(
                                dslice[:, :h2], dslice[:, :h2], skv[:, :h2])
                            nc.vector.tensor_add(
                                dslice[:, h2:], dslice[:, h2:], skv[:, h2:])
                        chunk_epilogue(dst_t, Co, e, rd, cr, dst_stack, qi=ci)
                        ci += 1
                        ro += cr

            # ---------------- f5 forward projection (from x5T) -------------
            def f5_forward(fwd):
                kt = fwd.kt
                nmm = 2 * OWN
                for wt in range(2):
                    for hb in range(OWN // 8):
                        ch = stm3.tile([128, 8 * 256], F16, tag="xtc")
                        nc.scalar.dma_start(ch[:], x5T[wt, hb])
                        for h in range(8):
                            for k in range(kt):
                                nc.tensor.matmul(
                                    fwd.pxf[:, k * 32:(k + 1) * 32],
                                    ch[:, h * 256 + k * 128:
                                       h * 256 + (k + 1) * 128],
                                    fb_sl(wt, hb * 8 + h),
                                    start=(fwd.i[k] == 0),
                                    stop=(fwd.i[k] == nmm - 1),
                                    skip_group_check=True)
                                fwd.i[k] += 1

            # ---------------- the network ----------------
            fw5 = FwdState("f5")
            f5_forward(fw5)
            load_late_consts()
            x5u = new_act("A", 9)
            fno_block("f5", None, x5u, dst_stack=1, fwd=fw5)
            x6a = new_act("B", 8)
            conv_stage("c6", 128, 0, 8, x5u, 9, x6a)
            x6b = new_act("A", 7)
            conv_stage("c6", 128, 1, 7, x6a, 8, x6b)
            x6 = new_dense("B", 128, _rows(6))
            fw6 = FwdState("f6")
            conv_stage("c6", 128, 2, 6, x6b, 7, x6, dense=True, fwd=fw6)
            x6u = new_act("A", 6)
            fno_block("f6", x6, x6u, dst_stack=2, fwd=fw6)
            x7a = new_act("B", 5)
            conv_stage("c7", 64, 0, 5, x6u, 6, x7a, dst_stack=2)
            x7b = new_act("A", 4)
            conv_stage("c7", 64, 1, 4, x7a, 5, x7b, dst_stack=2)
            x7 = new_dense("B", 64, _rows(3))
            fw7 = FwdState("f7")
            conv_stage("c7", 64, 2, 3, x7b, 4, x7, dense=True, fwd=fw7)
            x7u = new_act("A", 3)
            fno_block("f7", x7, x7u, dst_stack=3, fwd=fw7)
            x8a = new_act("B", 2)
            conv_stage("c8", 32, 0, 2, x7u, 3, x8a, dst_stack=3)
            x8 = new_dense("A", 32, _rows(1))
            fw8 = FwdState("f8")
            conv_stage("c8", 32, 1, 1, x8a, 2, x8, dense=True, fwd=fw8)
            x8u = new_act("B", 1)
            fno_block("f8", x8, x8u, dst_stack=3, fwd=fw8)
            x9 = new_dense("A", 16, OWN)
            conv_stage("c9", 16, 0, 0, x8u, 1, x9, dense=True)

            # final 1x1 conv (16 -> 2), owned rows only; 4-row chunks
            for g in range(OWN // 4):
                rd = 4 * g
                ps = fpsp.tile([2, 1024], F32, tag="fps", name="psO")
                for h in range(2):
                    nc.tensor.matmul(ps[:, h * 512:(h + 1) * 512], ow_t[:],
                                     x9[:16, rd * 256 + h * 512:
                                        rd * 256 + (h + 1) * 512],
                                     start=True, stop=True,
                                     skip_group_check=True)
                oc = stm2.tile([2, 1024], F32, tag="outch", name="outch")
                nc.scalar.activation(oc[:], ps[:], AF.Identity, bias=ob_t[:])
                nc.scalar.dma_start(out_sl[:, rd * WW:(rd + 4) * WW], oc[:])

    nc.compile()
    return nc


# ---------------------------------------------------------------------------
# host side
# ---------------------------------------------------------------------------

def _slice_rows(x, lo, hi):
    """x: (C, 256, 256) -> (C, hi-lo, 256) zero-padded out of range."""
    C = x.shape[0]
    out = np.zeros((C, hi - lo, WW), np.float32)
    a, b2 = max(lo, 0), min(hi, HH)
    if b2 > a:
        out[:, a - lo:b2 - lo] = x[:, a:b2]
    return out


def _host_inputs(inputs):
    i = inputs
    maps = []
    kk, ll = np.meshgrid(np.arange(M), np.arange(M), indexing="ij")
    kf = kk.reshape(-1).astype(np.float64)   # mu = 4k + l
    lf = ll.reshape(-1).astype(np.float64)
    alpha32 = np.where(lf == 0, 1.0, 2.0).repeat(2)  # per 32-comp row

    def basis(rows_abs, wvals):  # -> (32, len(rows), len(w))
        th = 2 * np.pi * (kf[:, None, None] * rows_abs[None, :, None] / HH
                          + lf[:, None, None] * wvals[None, None, :] / WW)
        out = np.empty((32, len(rows_abs), len(wvals)), np.float32)
        out[0::2] = np.cos(th) / 256.0
        out[1::2] = -np.sin(th) / 256.0
        return out

    # weights (identical on all cores)
    cw_np = {}
    cb_np = {}
    for tag, C, nst, _ in CONVS:
        w = np.asarray(i[tag + "_w"], np.float32)     # (n, co, ci, 3, 3)
        wt = w.transpose(0, 3, 4, 2, 1)               # (n, dy, dx, ci, co)
        if tag == "c6":
            cw_np[tag] = np.ascontiguousarray(wt.reshape(nst, 9, C, C))
        elif tag == "c7":
            cwv = np.zeros((nst, 6, 128, C), np.float32)
            for dx in range(3):
                cwv[:, dx, :C] = wt[:, 0, dx]         # dy=-1 via h0
                cwv[:, dx, C:2 * C] = wt[:, 1, dx]    # dy=0 via h1
                cwv[:, 3 + dx, C:2 * C] = wt[:, 2, dx]  # dy=+1 via h1
            cw_np[tag] = cwv
        else:  # c8, c9: 3-stack
            cwv = np.zeros((nst, 3, 3 * C, C), np.float32)
            for dx in range(3):
                for dy in range(3):
                    cwv[:, dx, dy * C:(dy + 1) * C] = wt[:, dy, dx]
            cw_np[tag] = cwv
        # device layout: (stage, i(pad 128), tap*C + o), contiguous load
        nst_, nt_, kw_, C_ = CWSHP[tag]
        cwi = np.zeros((nst_, 128, nt_ * C_), np.float32)
        cwi[:, :kw_, :] = cw_np[tag].transpose(0, 2, 1, 3).reshape(
            nst_, kw_, nt_ * C_)
        cw_np[tag] = cwi.astype(np.float16)
        cbv = np.zeros((128, nst), np.float32)
        cbv[:C] = np.asarray(i[tag + "_b"], np.float32).T
        cb_np[tag] = cbv
    fno_np = {}
    for tag, Ci, Co, e in FNOS:
        kt = (Ci + 127) // 128
        bw = np.asarray(i[tag + "_bw"], np.float32)[:, :, 0, 0]  # (oc, ic)
        full = np.ascontiguousarray(bw.T)                        # (ic, oc)
        bwT_ = np.zeros((128, kt * Co), np.float32)
        for k in range(kt):
            w = min(128, Ci - k * 128)
            bwT_[:w, k * Co:(k + 1) * Co] = full[k * 128:k * 128 + w]
        wr = np.asarray(i[tag + "_wr"], np.float32)   # (ic, oc, 4, 4)
        wi = np.asarray(i[tag + "_wi"], np.float32)
        # mixw layout: (128, kt*16*2*Co): slab for (k, mu) at (k*16+mu)*2*Co,
        # first Co cols = wr rows k*128.., next Co = wi rows
        mw = np.zeros((128, kt * 16 * 2 * Co), np.float32)
        for k in range(kt):
            w = min(128, Ci - k * 128)
            for mu in range(16):
                kk_, ll_ = mu // 4, mu % 4
                off = (k * 16 + mu) * 2 * Co
                mw[:w, off:off + Co] = wr[k * 128:k * 128 + w, :, kk_, ll_]
                mw[:w, off + Co:off + 2 * Co] = wi[k * 128:k * 128 + w, :, kk_, ll_]
        bbv = np.zeros((128, 1), np.float32)
        bbv[:Co, 0] = np.asarray(i[tag + "_bb"], np.float32)
        fno_np[tag] = (bwT_.astype(np.float16), bbv,
                       (mw * 4096.0).astype(np.float16))
    owT_np = np.ascontiguousarray(
        np.asarray(i["ow"], np.float32)[:, :, 0, 0].T).astype(np.float16)
    ob_np = np.asarray(i["ob"], np.float32)[:, None]

    skips_full = {"f5": np.asarray(i["x4"], np.float32),
                  "f6": np.asarray(i["x3"], np.float32),
                  "f7": np.asarray(i["x2"], np.float32),
                  "f8": np.asarray(i["x1"], np.float32)}
    x5 = np.asarray(i["x5"], np.float32)
    wvals = np.arange(WW, dtype=np.float64)
    K_rows9 = _rows(9)

    for core in range(NCORES):
        b, q = divmod(core, 4)
        r0 = OWN * q
        m = {}
        m["x5b_sl"] = _slice_rows(x5[b], r0 - 9, r0 + OWN + 9).reshape(256, -1).astype(np.float16)
        m["x4_sl"] = _slice_rows(skips_full["f5"][b], r0 - 9, r0 + OWN + 9).reshape(128, -1).astype(np.float16)
        m["x3_sl"] = _slice_rows(skips_full["f6"][b], r0 - 6, r0 + OWN + 6).reshape(64, -1).astype(np.float16)
        m["x2_sl"] = _slice_rows(skips_full["f7"][b], r0 - 3, r0 + OWN + 3).reshape(32, -1).astype(np.float16)
        m["x1_sl"] = _slice_rows(skips_full["f8"][b], r0 - 1, r0 + OWN + 1).reshape(16, -1).astype(np.float16)
        xo = x5[b][:, r0:r0 + OWN, :]                       # (256c, 64h, 256w)
        x5T_ = xo.transpose(1, 2, 0).reshape(OWN, 2, 128, 256).transpose(1, 0, 2, 3)
        # (wt, h, w, c) -> (wt, hb, w, (h8 c)) for contiguous device loads
        x5T_ = (x5T_.reshape(2, 8, 8, 128, 256).transpose(0, 1, 3, 2, 4)
                .reshape(2, 8, 128, 8 * 256))
        m["x5T"] = np.ascontiguousarray(x5T_).astype(np.float16)
        # fwd basis (w, (wt h m)) at abs rows r0+h, col wt*128+w
        fbb = basis(np.arange(r0, r0 + OWN, dtype=np.float64), wvals)  # (32,64,256)
        fbb = (fbb.transpose(2, 1, 0)                       # (w256, h, m)
               .reshape(2, 128, OWN, 32)                    # (wt, w, h, m)
               .transpose(1, 0, 2, 3)                       # (w, wt, h, m)
               .reshape(128, 2 * OWN * 32))
        m["fb"] = np.ascontiguousarray(fbb).astype(np.float16)
        rows = np.arange(r0 - 9, r0 + OWN + 9, dtype=np.float64)
        gbb = basis(rows, wvals) * alpha32[:, None, None]
        gbb[:, (rows < 0) | (rows >= HH), :] = 0.0
        m["gb"] = gbb.reshape(32, -1).astype(np.float16)
        mrow = ((rows >= 0) & (rows < HH)).astype(np.float32)
        m["mask_pp"] = np.tile(mrow[None, :], (128, 1)).astype(np.float16)
        bs = np.zeros((128, 2), np.float32)
        bs[:, b] = 1.0
        m["bsel"] = bs
        m["zpadb"] = np.zeros((128, 2 * K_rows9), np.float16)
        for tag, C, nst, _ in CONVS:
            m[tag + "w"] = cw_np[tag]
            m[tag + "b"] = cb_np[tag]
        for tag, Ci, Co, e in FNOS:
            bwT_, bb_, mw_ = fno_np[tag]
            m[tag + "_bwT"] = bwT_
            m[tag + "_bb"] = bb_
            m[tag + "_mixw"] = mw_
        m["owT"] = owT_np
        m["ob"] = ob_np
        maps.append(m)
    return maps


_NC_CACHE = {}


def kernel(**inputs):
    if "nc" not in _NC_CACHE:
        _NC_CACHE["nc"] = _build_nc()
    nc = _NC_CACHE["nc"]
    maps = _host_inputs(inputs)
    res = run_bass_kernel_spmd(nc, maps, list(range(NCORES)), trace=False)
    out = np.zeros((B, 2, HH, WW), np.float32)
    for core in range(NCORES):
        b, q = divmod(core, 4)
        r0 = OWN * q
        out[b, :, r0:r0 + OWN, :] = res.results[core]["out_sl"].reshape(2, OWN, WW)
    return out
